# revision 52
# baseline (speedup 1.0000x reference)
"""Trainium2 Bass kernel for Conv2DCaps with dynamic routing (3 iterations).

Sharding: 8 cores = batch(4) x H-halves(2). Each core gets a 21-row slab of
its batch image (4-row halo overlap) and computes the full routing locally;
the host extracts the exact 15-row output half from each core.

Device layout ("transposed world"): features on partitions, positions on the
free dim. Key tensors per core (grid 21x32 -> P=672 positions):
  u[rc]   [jm=128, (i=8, p=672)]  fp16, SBUF-resident (built once on PE)
  bT      [(rc,i,j)=576 rows -> 5 tiles of [128, 672]] fp32 routing logits
  c       same row layout as bT, fp16 softmax coefficients
All contractions (u-build, iter0-d, sum-over-m, softmax row-sums, expansions)
run on the PE via small/masked matmuls; products run on DVE at fp16 2x mode;
coefficient broadcast (8 rows -> 128 partitions) rides on DMA engines.
"""
import numpy as np

import concourse.bass as bass
import concourse.bacc as bacc
import concourse.mybir as mybir
import concourse.tile as tile
from concourse.bass_utils import run_bass_kernel_spmd

FP32 = mybir.dt.float32
FP16 = mybir.dt.float16
AF = mybir.ActivationFunctionType
OP = mybir.AluOpType

KH = KW = 3
RC = KH * KW                  # 9
B, H, W, CI, NI = 4, 32, 32, 8, 16
CJ, NJ = 8, 16
HJ = WJ = 30
EPS = 1e-7
J_ALL = float(HJ * WJ * CJ)           # 7200
J_ADD = J_ALL - KH * KW * CJ          # 7128
R_NUM = 3

RROWS = 21                    # rows per shard
P = RROWS * W                 # 672 positions
QR, QC = RROWS - 2, 30        # 19 x 30 local outputs
Q = QR * QC                   # 570
PGR, PGC = QR + 4, 34         # padded v grid 23 x 34
NBT = 5                       # bT tiles (576 rows)


class _PhaseStop(Exception):
    pass


def _build_program(phases=99):
    nc = bacc.Bacc("TRN2", target_bir_lowering=False)

    xTn = nc.declare_dram_parameter("xTn", [128, 2 * P], FP16, isOutput=False)
    xT128 = nc.declare_dram_parameter("xT128", [128, P], FP16, isOutput=False)
    wdn = nc.declare_dram_parameter("wdn", [128, 18 * 128], FP16, isOutput=False)
    wd = nc.declare_dram_parameter("wd", [128, RC * 128], FP16, isOutput=False)
    mones = nc.declare_dram_parameter("mones", [128, 8 * 64], FP16, isOutput=False)
    msum = nc.declare_dram_parameter("msum", [128, NBT * 8], FP16, isOutput=False)
    dltr = nc.declare_dram_parameter("dltr", [8, NBT * 128], FP16, isOutput=False)
    ones16 = nc.declare_dram_parameter("ones16", [128, 8], FP16, isOutput=False)
    dltj = nc.declare_dram_parameter("dltj", [8, 128], FP16, isOutput=False)
    e128 = nc.declare_dram_parameter("e128", [128, 512], FP16, isOutput=False)
    vout = nc.declare_dram_parameter("vout", [128, Q], FP32, isOutput=True)

    with tile.TileContext(nc) as tc:
        with (
            tc.tile_pool(name="const", bufs=1) as cpool,
            tc.tile_pool(name="ubig", bufs=1) as upool,
            tc.tile_pool(name="state", bufs=1) as spool,
            tc.tile_pool(name="work", bufs=2) as wpool,
            tc.tile_pool(name="tiny", bufs=3) as tpool,
            tc.tile_pool(name="ex", bufs=1) as epool,
            tc.tile_pool(name="cep", bufs=4) as cppool,
            tc.tile_pool(name="psA", bufs=3, space="PSUM") as ppA,
            tc.tile_pool(name="psB", bufs=1, space="PSUM") as ppB,
        ):
            # ---- load constants / inputs ----
            t_xtn = cpool.tile([128, 2 * P], FP16, tag="xtn")
            t_x128 = cpool.tile([128, P], FP16, tag="x128")
            t_wdn = cpool.tile([128, 18 * 128], FP16, tag="wdn")
            t_wd = cpool.tile([128, RC * 128], FP16, tag="wd")
            t_mo = cpool.tile([128, 8 * 64], FP16, tag="mo")
            t_ms = cpool.tile([128, NBT * 8], FP16, tag="ms")
            t_dr = cpool.tile([8, NBT * 128], FP16, tag="dr")
            t_o16 = cpool.tile([128, 8], FP16, tag="o16")
            t_dj = cpool.tile([8, 128], FP16, tag="dj")
            t_e128 = cpool.tile([128, 512], FP16, tag="e128")
            t_eps = cpool.tile([128, 1], FP32, tag="eps")
            t_dum = cpool.tile([8, 1], FP32, tag="dum")
            t_shm8 = cpool.tile([128, 1], FP32, tag="shm8")
            nc.vector.memset(t_shm8[:], -8.0)
            nc.vector.memset(t_eps[:], EPS)
            # prefetch the Sqrt act table while everything else loads
            nc.scalar.activation(t_dum[:], t_eps[0:8, :], AF.Sqrt)
            # spread input loads over 3 queues; s0 needs x128+wd first
            for k, (t, src) in enumerate(((t_x128, xT128), (t_wd, wd),
                                          (t_xtn, xTn), (t_wdn, wdn),
                                          (t_o16, ones16), (t_dj, dltj),
                                          (t_mo, mones), (t_ms, msum),
                                          (t_dr, dltr), (t_e128, e128))):
                eng = (nc.sync, nc.scalar, nc.gpsimd)[k % 3]
                eng.dma_start(t[:], src[:])

            # ---- persistent state ----
            t_u = [upool.tile([128, 8 * P], FP16, tag=f"u{rc}", name=f"u{rc}") for rc in range(RC)]
            t_bt = [spool.tile([128, P], FP32, tag=f"bt{t}", name=f"bt{t}") for t in range(NBT)]
            t_c = [spool.tile([128, P], FP16, tag=f"c{t}", name=f"c{t}") for t in range(NBT)]
            t_vp = [spool.tile([128, PGR, PGC], FP16, tag=f"vp{c}", name=f"vp{c}") for c in range(KW)]
            t_s = spool.tile([128, Q], FP16, tag="s")

            for t in t_bt:
                nc.vector.memset(t[:], 0.0)
            for t in t_vp:
                nc.gpsimd.memset(t[:], 0.0)

            cp_flip = [0]

            def psum_to_sbuf(dst_ap, src_ap):
                # ACT-heavy rotation for PSUM-exit copies: DVE is the
                # kernel-wide bottleneck, ACT has slack
                if cp_flip[0] % 3 == 0:
                    nc.vector.tensor_copy(dst_ap, src_ap)
                else:
                    nc.scalar.activation(dst_ap, src_ap, AF.Copy)
                cp_flip[0] += 1

            # ---- u-build: u[rc][:, i*P:(i+1)*P] = wdn-block.T @ xTn-block ----
            def u_build():
                for rc in range(RC):
                    for i in range(CI):
                        b = rc * 8 + i
                        rg = 32 * (b % 4)
                        lhsT = t_wdn[rg:rg + 16,
                                     128 * (b // 4):128 * (b // 4) + 128]
                        rhs = t_xtn[rg:rg + 16, P * (i // 4):P * (i // 4) + P]
                        ps = ppA.tile([128, P], FP32, tag="ps", name="ps")
                        nc.tensor.matmul(ps[:, 0:512], lhsT, rhs[:, 0:512],
                                         start=True, stop=True,
                                         tile_position=(rg, 0))
                        nc.tensor.matmul(ps[:, 512:P], lhsT, rhs[:, 512:P],
                                         start=True, stop=True,
                                         tile_position=(rg, 0))
                        psum_to_sbuf(t_u[rc][:, i * P:(i + 1) * P], ps[:])

            def s_accum(rc, d_grid_ap):
                """t_s[jm, q] += window of d (d viewed as [128, 21, 32])."""
                r, c = divmod(rc, KW)
                win = d_grid_ap[:, r:r + QR, c:c + QC]
                sview = t_s[:].rearrange("p (a b) -> p a b", b=QC)
                if rc == 0:
                    nc.vector.tensor_copy(sview, win)
                else:
                    with nc.allow_low_precision(reason="fp16 s accumulate"):
                        nc.vector.tensor_tensor(sview, sview, win, OP.add)

            def squeeze(it):
                """v = squeeze(s); writes vpads (it<2) or vout (it==2)."""
                # s*s overflows fp16 at later iterations: compute (s*sig)*s so
                # the fp16 products stay in range, and un-scale via 1/sig.
                sig = (1.0, 2.0 ** -8, 2.0 ** -12)[it]
                isig = 1.0 / sig
                s2 = wpool.tile([128, Q], FP16, tag="s2")
                nc.vector.scalar_tensor_tensor(s2[:], t_s[:], sig, t_s[:],
                                               OP.mult, OP.mult)
                sq_t = ppA.tile([128, P], FP32, tag="ps", name="sq_t")
                nc.tensor.matmul(sq_t[0:8, 0:512], t_o16[:], s2[:, 0:512],
                                 start=True, stop=True)
                nc.tensor.matmul(sq_t[0:8, 512:Q], t_o16[:], s2[:, 512:Q],
                                 start=True, stop=True)
                sq_ap = sq_t[0:8, 0:Q]        # = sig * true_sq
                t1 = tpool.tile([8, P], FP32, tag="tmp8")
                nc.vector.tensor_scalar(t1[:, 0:Q], sq_ap, isig, 1.0,
                                        OP.mult, OP.add)
                sqr = tpool.tile([8, P], FP32, tag="tmp8")
                nc.scalar.activation(sqr[:, 0:Q], sq_ap, AF.Sqrt, scale=isig,
                                     bias=t_eps[0:8, :])
                if it < R_NUM - 1:
                    # prefetch the Exp table for the upcoming softmax
                    nc.scalar.activation(t_dum[:], t_eps[0:8, :], AF.Exp)
                den = tpool.tile([8, P], FP32, tag="tmp8")
                nc.vector.tensor_tensor(den[:, 0:Q], t1[:, 0:Q], sqr[:, 0:Q],
                                        OP.mult)
                rec = tpool.tile([8, P], FP32, tag="tmp8")
                nc.vector.reciprocal_approx_fast(rec[:, 0:Q], den[:, 0:Q])
                f = tpool.tile([8, P], FP16, tag="tmp8f")
                nc.vector.scalar_tensor_tensor(f[:, 0:Q], sq_ap, isig,
                                               rec[:, 0:Q], OP.mult, OP.mult)
                fe_t = ppA.tile([128, P], FP32, tag="ps", name="fe_t")
                nc.tensor.matmul(fe_t[:, 0:512], t_dj[:], f[:, 0:512],
                                 start=True, stop=True)
                nc.tensor.matmul(fe_t[:, 512:Q], t_dj[:], f[:, 512:Q],
                                 start=True, stop=True)
                fe_ap = fe_t[:, 0:Q]
                if it == R_NUM - 1:
                    t_vo = wpool.tile([128, Q], FP32, tag="vo", name="vo2")
                    nc.vector.tensor_tensor(t_vo[:], t_s[:], fe_ap, OP.mult)
                    # split the store across 4 queues (128 one-row
                    # descriptors on one queue would cost ~9us)
                    for q, eng in enumerate((nc.sync, nc.scalar, nc.gpsimd,
                                             nc.sync)):
                        eng.dma_start(vout[32 * q:32 * (q + 1), :],
                                      t_vo[32 * q:32 * (q + 1), :])
                else:
                    for c in range(KW):
                        dst = t_vp[c][:, 2:2 + QR, c:c + QC]
                        nc.vector.tensor_tensor(
                            dst,
                            t_s[:].rearrange("p (a b) -> p a b", b=QC),
                            fe_ap.rearrange("p (a b) -> p a b", b=QC),
                            OP.mult)

            def agreement(nxt):
                """bT += sum_m u*vp per (rc, i); PE masked-ones matmuls.

                rc pairs (2t, 2t+1) land in rows 0:64 / 64:128 of one PSUM
                tile so each bT tile updates with a single 128-row add. The
                next softmax's exp (ACT) + row-sum (PE) for tile t are
                emitted as soon as bT[t] is final, overlapping phases."""
                shift = 0.0 if nxt < 2 else -8.0
                exs = [None] * NBT
                sume_t = ppB.tile([128, 1024], FP32, tag="sume")
                dps = None
                for rc in range(RC):
                    r, c = divmod(rc, KW)
                    t, row = rc // 2, (rc % 2) * 64
                    if row == 0:
                        dps = ppA.tile([128, P], FP32, tag="ps",
                                       name=f"dps{t}")
                    vslice = t_vp[c][:, 2 - r:2 - r + RROWS, 0:W]
                    vb = vslice.unsqueeze(1).broadcast_to([128, 4, RROWS, W])
                    for h in range(2):
                        prod = wpool.tile([128, 4 * P], FP16, tag="big",
                                          name=f"prod{rc}_{h}")
                        useg = t_u[rc][:, h * 4 * P:(h + 1) * 4 * P]
                        nc.vector.tensor_tensor(
                            prod[:].rearrange("p (i a b) -> p i a b", i=4, b=W),
                            useg.rearrange("p (i a b) -> p i a b", i=4, b=W),
                            vb, OP.mult)
                        for ih in range(4):
                            i = h * 4 + ih
                            lhsT = t_mo[:, i * 64:(i + 1) * 64]
                            seg = prod[:, ih * P:(ih + 1) * P]
                            nc.tensor.matmul(dps[row:row + 64, 0:512], lhsT,
                                             seg[:, 0:512],
                                             start=(i == 0), stop=(i == CI - 1))
                            nc.tensor.matmul(dps[row:row + 64, 512:P], lhsT,
                                             seg[:, 512:P],
                                             start=(i == 0), stop=(i == CI - 1))
                    if row == 64 or rc == RC - 1:
                        rows = 64 if rc == RC - 1 else 128
                        bslice = t_bt[t][0:rows, :]
                        nc.vector.tensor_tensor(bslice, bslice,
                                                dps[0:rows, :], OP.add)
                        ex = epool.tile([128, P], FP16, tag=f"ex{t}",
                                        name=f"ex{t}")
                        if shift == 0.0:
                            nc.scalar.activation(ex[:], t_bt[t][:], AF.Exp,
                                                 scale=8.0)
                        else:
                            nc.scalar.activation(ex[:], t_bt[t][:], AF.Exp,
                                                 scale=8.0, bias=t_shm8[:])
                        exs[t] = ex
                        lhsT = t_ms[:, t * 8:(t + 1) * 8]
                        nc.tensor.matmul(sume_t[0:8, 0:512], lhsT,
                                         ex[:, 0:512],
                                         start=(t == 0), stop=(t == NBT - 1))
                        nc.tensor.matmul(sume_t[0:8, 512:P], lhsT,
                                         ex[:, 512:P],
                                         start=(t == 0), stop=(t == NBT - 1))
                return exs, sume_t

            def softmax_rest_tile(it, exs, sume_t, t):
                """Per-tile tail of the softmax: c[t] = ex[t] * re[t].

                c = exp(8 bT) * 7200 / (sum + eps + 7128); exp + row-sums
                already ran inside agreement(). A constant logit shift keeps
                exp in fp16 range (it==2 logits reach ~17); it cancels
                exactly in c = 7200*ex/(sum + 7128*e^shift)."""
                if t == 0:
                    shift = 0.0 if it < 2 else -8.0
                    cadd = (J_ADD + EPS) * float(np.exp(shift))
                    # prefetch the Sqrt table for the upcoming squeeze
                    nc.scalar.activation(t_dum[:], t_eps[0:8, :], AF.Sqrt)
                    sumb = tpool.tile([8, P], FP32, tag="tmp8")
                    nc.vector.tensor_scalar_add(sumb[:], sume_t[0:8, 0:P],
                                                cadd)
                    rec = tpool.tile([8, P], FP32, tag="tmp8")
                    nc.vector.reciprocal_approx_fast(rec[:], sumb[:])
                    rec16 = tpool.tile([8, P], FP16, tag="tmp8f",
                                       name="rec16")
                    softmax_rest_tile.rec16 = rec16
                    nc.vector.tensor_copy(rec16[:], rec[:])
                rec16 = softmax_rest_tile.rec16
                re = ppA.tile([128, P], FP32, tag="ps", name="re")
                lhsT = t_dr[:, t * 128:(t + 1) * 128]
                nc.tensor.matmul(re[:, 0:512], lhsT, rec16[:, 0:512],
                                 start=True, stop=True)
                nc.tensor.matmul(re[:, 512:P], lhsT, rec16[:, 512:P],
                                 start=True, stop=True)
                # ACT drains re to fp16 so the c-mult runs at DVE 2x
                re16 = cppool.tile([128, P], FP16, tag="ce16", name="re16")
                nc.scalar.activation(re16[:], re[:], AF.Copy)
                nc.vector.tensor_tensor(t_c[t][:], exs[t][:], re16[:],
                                        OP.mult)

            def expand_unit(rc, i, ce_ps):
                """PE-expand c rows for slot i into a [128, P] PSUM tile."""
                t, row0 = rc // 2, (rc % 2) * 64
                r0 = row0 + i * 8
                r32 = (r0 // 32) * 32           # 32-aligned ldweights base
                g = (r0 - r32) // 8             # 8-row group within window
                lhsT = t_e128[r32:r32 + 32, g * 128:(g + 1) * 128]
                rhs = t_c[t][r32:r32 + 32, :]
                nc.tensor.matmul(ce_ps[:, 0:512], lhsT, rhs[:, 0:512],
                                 start=True, stop=True,
                                 tile_position=(r32, 0))
                nc.tensor.matmul(ce_ps[:, 512:P], lhsT, rhs[:, 512:P],
                                 start=True, stop=True,
                                 tile_position=(r32, 0))

            # per-i slot kinds in the d-pass:
            #  'a': ACT drains PSUM->fp16 SBUF, DVE fp16-2x product
            #  'd': DVE multiplies straight from PSUM (1x, but no drain op)
            #  'g': ACT drains, GpSimd does the product (PSUM is gp-illegal)
            CE_SLOT = ("a", "a", "a", "a", "g", "a", "a", "a")

            def d_pass_coeff(it, exs, sume_t):
                """s = sum_rc window(sum_i c*u): PE expands c rows (j -> jm)
                via a constant delta-matrix matmul into PSUM; ACT drains to
                fp16 SBUF; DVE fp16-2x products + in-place add tree. The
                per-tile softmax tail is interleaved so expansions for
                rc-pair t start right after c[t] is ready."""
                for rc in range(RC):
                    if rc % 2 == 0:
                        softmax_rest_tile(it, exs, sume_t, rc // 2)
                    cu = wpool.tile([128, 8 * P], FP16, tag="cu",
                                    name=f"cu{rc}")
                    for i in range(CI):
                        ce_ps = ppA.tile([128, P], FP32, tag="ps", name="ceps")
                        expand_unit(rc, i, ce_ps)
                        useg = t_u[rc][:, i * P:(i + 1) * P]
                        cuseg = cu[:, i * P:(i + 1) * P]
                        kind = CE_SLOT[i]
                        if kind == "d":
                            nc.vector.tensor_tensor(cuseg, useg, ce_ps[:],
                                                    OP.mult)
                        else:
                            ce16 = cppool.tile([128, P], FP16, tag="ce16")
                            nc.scalar.activation(ce16[:], ce_ps[:], AF.Copy)
                            eng = nc.gpsimd if kind == "g" else nc.vector
                            eng.tensor_tensor(cuseg, useg, ce16[:], OP.mult)
                    with nc.allow_low_precision(reason="fp16 routing tree"):
                        eng1 = nc.gpsimd if rc % 2 == 0 else nc.vector
                        eng1.tensor_tensor(cu[:, 0:4 * P], cu[:, 0:4 * P],
                                           cu[:, 4 * P:8 * P], OP.add)
                        nc.vector.tensor_tensor(cu[:, 0:2 * P], cu[:, 0:2 * P],
                                                cu[:, 2 * P:4 * P], OP.add)
                        nc.vector.tensor_tensor(cu[:, 0:P], cu[:, 0:P],
                                                cu[:, P:2 * P], OP.add)
                    s_accum(rc, cu[:, 0:P].rearrange("p (a b) -> p a b", b=W))

            if phases < 1:
                raise _PhaseStop
            # ================= iteration 0 =================
            # c == 1 exactly (b=0): s0 = sum_rc window(Wd[rc].T @ xT128)
            # fully on PE via strided window views of xT128, PSUM-accumulated
            # over all 9 taps (split into two banks: 10+9 output rows).
            # Emitted before the u-build so squeeze(0) overlaps it.
            xv = t_x128[:].rearrange("p (a b) -> p a b", b=W)
            s0 = ppB.tile([128, 1024], FP32, tag="sume")
            s0a = s0[:, 0:10 * QC]          # bank 0
            s0b = s0[:, 512:512 + 9 * QC]   # bank 1
            for rc in range(RC):
                r, c = divmod(rc, KW)
                lhsT = t_wd[:, rc * 128:(rc + 1) * 128]
                rhs_a = xv[:, r:r + 10, c:c + QC]
                rhs_b = xv[:, r + 10:r + QR, c:c + QC]
                nc.tensor.matmul(s0a, lhsT, rhs_a,
                                 start=(rc == 0), stop=(rc == RC - 1))
                nc.tensor.matmul(s0b, lhsT, rhs_b,
                                 start=(rc == 0), stop=(rc == RC - 1))
            nc.vector.tensor_copy(t_s[:, 0:10 * QC], s0a)
            nc.vector.tensor_copy(t_s[:, 10 * QC:Q], s0b)
            squeeze(0)
            if phases < 2:
                raise _PhaseStop
            u_build()
            exs, sume_t = agreement(1)
            if phases < 3:
                raise _PhaseStop

            # ================= iterations 1..2 =================
            for it in range(1, R_NUM):
                if phases < 3 + (it - 1) * 4 + 1:
                    raise _PhaseStop
                d_pass_coeff(it, exs, sume_t)
                squeeze(it)
                if phases < 3 + (it - 1) * 4 + 3:
                    raise _PhaseStop
                if it < R_NUM - 1:
                    exs, sume_t = agreement(2)

    nc.compile()
    return nc


_PROGRAM = None


def _get_program():
    global _PROGRAM
    if _PROGRAM is None:
        _PROGRAM = _build_program()
    return _PROGRAM


def _host_inputs(x, w):
    """Build the 8 per-core input maps."""
    wdn_p = np.zeros((128, 18 * 128), np.float16)
    wd_p = np.zeros((128, RC * 128), np.float16)
    wf = w.reshape(RC, CI, NI, CJ * NJ)              # [rc, i, n, jm]
    for rc in range(RC):
        for i in range(CI):
            b = rc * 8 + i
            wdn_p[32 * (b % 4):32 * (b % 4) + 16,
                  128 * (b // 4):128 * (b // 4) + 128] = wf[rc, i]
            wd_p[i * 16:(i + 1) * 16, rc * 128:(rc + 1) * 128] = wf[rc, i]
    mones = np.zeros((128, 8 * 64), np.float16)
    for i in range(CI):
        for j in range(CJ):
            mones[j * 16:(j + 1) * 16, i * 64 + i * 8 + j] = 1.0
    msum = np.zeros((128, NBT * 8), np.float16)
    dltr = np.zeros((8, NBT * 128), np.float16)
    for g in range(RC * CI * CJ):                    # g = rc*64 + i*8 + j
        i = (g % 64) // 8
        t, r = g // 128, g % 128
        msum[r, t * 8 + i] = 1.0
        dltr[i, t * 128 + r] = J_ALL
    e128p = np.zeros((128, 512), np.float16)
    for k in range(128):
        g = (k % 32) // 8
        j = k % 8
        e128p[k, g * 128 + j * 16:g * 128 + (j + 1) * 16] = 1.0
    ones16 = np.zeros((128, 8), np.float16)
    dltj = np.zeros((8, 128), np.float16)
    for j in range(CJ):
        ones16[j * 16:(j + 1) * 16, j] = 1.0
        dltj[j, j * 16:(j + 1) * 16] = 1.0

    shared = dict(wdn=wdn_p, wd=wd_p, mones=mones, msum=msum, dltr=dltr,
                  ones16=ones16, dltj=dltj, e128=e128p)

    in_maps = []
    for core in range(8):
        b, half = divmod(core, 2)
        r0 = 0 if half == 0 else H - RROWS
        xs = x[b, r0:r0 + RROWS].astype(np.float16)   # (21, 32, 8, 16)
        xt128 = np.ascontiguousarray(xs.reshape(P, CI * NI).T)
        xtn = np.zeros((128, 2 * P), np.float16)
        for i in range(CI):
            xtn[32 * (i % 4):32 * (i % 4) + 16,
                P * (i // 4):P * (i // 4) + P] = xs[:, :, i, :].reshape(P, NI).T
        m = dict(shared)
        m["xTn"] = xtn
        m["xT128"] = xt128
        in_maps.append(m)
    return in_maps


def _assemble(results):
    out = np.zeros((B, HJ, WJ, CJ, NJ), np.float32)
    for core, res in enumerate(results):
        b, half = divmod(core, 2)
        v = res["vout"].reshape(CJ, NJ, QR, QC).transpose(2, 3, 0, 1)
        if half == 0:
            out[b, 0:15] = v[0:15]
        else:
            out[b, 15:30] = v[4:19]
    return out


def run(x, w, trace=False):
    x = np.asarray(x, np.float32)
    w = np.asarray(w, np.float32)
    nc = _get_program()
    in_maps = _host_inputs(x, w)
    res = run_bass_kernel_spmd(nc, in_maps, core_ids=list(range(8)), trace=trace)
    return _assemble(res.results), res


def kernel(x, w):
    out, _ = run(x, w)
    return out



# revision 54
# speedup vs baseline: 1.1448x; 1.1448x over previous
"""Trainium2 Bass kernel for Conv2DCaps with dynamic routing (3 iterations).

Sharding: 8 cores = batch(4) x H-halves(2). Each core gets a 21-row slab of
its batch image (4-row halo overlap) and computes the full routing locally;
the host extracts the exact 15-row output half from each core.

Device layout ("transposed world"): features on partitions, positions on the
free dim. Key tensors per core (grid 21x32 -> P=672 positions):
  u[rc]   [jm=128, (i=8, p=672)]  fp16, SBUF-resident (built once on PE)
  bT      [(rc,i,j)=576 rows -> 5 tiles of [128, 672]] fp32 routing logits
  c       same row layout as bT, fp16 softmax coefficients
All contractions (u-build, iter0-d, sum-over-m, softmax row-sums, expansions)
run on the PE via small/masked matmuls; products run on DVE at fp16 2x mode;
coefficient broadcast (8 rows -> 128 partitions) rides on DMA engines.
"""
import numpy as np

import concourse.bass as bass
import concourse.bacc as bacc
import concourse.mybir as mybir
import concourse.tile as tile
from concourse.bass_utils import run_bass_kernel_spmd

FP32 = mybir.dt.float32
FP16 = mybir.dt.float16
AF = mybir.ActivationFunctionType
OP = mybir.AluOpType

KH = KW = 3
RC = KH * KW                  # 9
B, H, W, CI, NI = 4, 32, 32, 8, 16
CJ, NJ = 8, 16
HJ = WJ = 30
EPS = 1e-7
J_ALL = float(HJ * WJ * CJ)           # 7200
J_ADD = J_ALL - KH * KW * CJ          # 7128
R_NUM = 3

RROWS = 21                    # rows per shard
P = RROWS * W                 # 672 positions
QR, QC = RROWS - 2, 30        # 19 x 30 local outputs
Q = QR * QC                   # 570
PGR, PGC = QR + 4, 34         # padded v grid 23 x 34
NBT = 5                       # bT tiles (576 rows)


class _PhaseStop(Exception):
    pass


def _build_program(phases=99):
    nc = bacc.Bacc("TRN2", target_bir_lowering=False)

    xTn = nc.declare_dram_parameter("xTn", [128, 2 * P], FP16, isOutput=False)
    xT128 = nc.declare_dram_parameter("xT128", [128, P], FP16, isOutput=False)
    wdn = nc.declare_dram_parameter("wdn", [128, 18 * 128], FP16, isOutput=False)
    wd = nc.declare_dram_parameter("wd", [128, RC * 128], FP16, isOutput=False)
    mones = nc.declare_dram_parameter("mones", [128, 8 * 64], FP16, isOutput=False)
    msum = nc.declare_dram_parameter("msum", [128, NBT * 8], FP16, isOutput=False)
    dltr = nc.declare_dram_parameter("dltr", [8, NBT * 128], FP16, isOutput=False)
    ones16 = nc.declare_dram_parameter("ones16", [128, 8], FP16, isOutput=False)
    dltj = nc.declare_dram_parameter("dltj", [8, 128], FP16, isOutput=False)
    e128 = nc.declare_dram_parameter("e128", [128, 512], FP16, isOutput=False)
    vout = nc.declare_dram_parameter("vout", [128, Q], FP32, isOutput=True)

    with tile.TileContext(nc) as tc:
        with (
            tc.tile_pool(name="const", bufs=1) as cpool,
            tc.tile_pool(name="ubig", bufs=1) as upool,
            tc.tile_pool(name="state", bufs=1) as spool,
            tc.tile_pool(name="work", bufs=2) as wpool,
            tc.tile_pool(name="tiny", bufs=3) as tpool,
            tc.tile_pool(name="ex", bufs=1) as epool,
            tc.tile_pool(name="cep", bufs=4) as cppool,
            tc.tile_pool(name="psA", bufs=3, space="PSUM") as ppA,
            tc.tile_pool(name="psB", bufs=1, space="PSUM") as ppB,
        ):
            # ---- load constants / inputs ----
            t_xtn = cpool.tile([128, 2 * P], FP16, tag="xtn")
            t_x128 = cpool.tile([128, P], FP16, tag="x128")
            t_wdn = cpool.tile([128, 18 * 128], FP16, tag="wdn")
            t_wd = cpool.tile([128, RC * 128], FP16, tag="wd")
            t_mo = cpool.tile([128, 8 * 64], FP16, tag="mo")
            t_ms = cpool.tile([128, NBT * 8], FP16, tag="ms")
            t_dr = cpool.tile([8, NBT * 128], FP16, tag="dr")
            t_o16 = cpool.tile([128, 8], FP16, tag="o16")
            t_dj = cpool.tile([8, 128], FP16, tag="dj")
            t_e128 = cpool.tile([128, 512], FP16, tag="e128")
            t_eps = cpool.tile([128, 1], FP32, tag="eps")
            t_dum = cpool.tile([8, 1], FP32, tag="dum")
            t_shm8 = cpool.tile([128, 1], FP32, tag="shm8")
            nc.vector.memset(t_shm8[:], -8.0)
            nc.vector.memset(t_eps[:], EPS)
            # prefetch the Sqrt act table while everything else loads
            nc.scalar.activation(t_dum[:], t_eps[0:8, :], AF.Sqrt)
            # spread input loads over 3 queues; s0 needs x128+wd first
            for k, (t, src) in enumerate(((t_x128, xT128), (t_wd, wd),
                                          (t_xtn, xTn), (t_wdn, wdn),
                                          (t_o16, ones16), (t_dj, dltj),
                                          (t_mo, mones), (t_ms, msum),
                                          (t_dr, dltr), (t_e128, e128))):
                eng = (nc.sync, nc.scalar, nc.gpsimd)[k % 3]
                eng.dma_start(t[:], src[:])

            # ---- persistent state ----
            t_u = [upool.tile([128, 8 * P], FP16, tag=f"u{rc}", name=f"u{rc}") for rc in range(RC)]
            t_bt = [spool.tile([128, P], FP32, tag=f"bt{t}", name=f"bt{t}") for t in range(NBT)]
            t_c = [spool.tile([128, P], FP16, tag=f"c{t}", name=f"c{t}") for t in range(NBT)]
            t_vp = [spool.tile([128, PGR, PGC], FP16, tag=f"vp{c}", name=f"vp{c}") for c in range(KW)]
            t_s = spool.tile([128, Q], FP16, tag="s")

            for t in t_bt:
                nc.vector.memset(t[:], 0.0)
            for t in t_vp:
                nc.gpsimd.memset(t[:], 0.0)

            cp_flip = [0]

            def psum_to_sbuf(dst_ap, src_ap):
                # ACT-heavy rotation for PSUM-exit copies: DVE is the
                # kernel-wide bottleneck, ACT has slack
                if cp_flip[0] % 3 == 0:
                    nc.vector.tensor_copy(dst_ap, src_ap)
                else:
                    nc.scalar.activation(dst_ap, src_ap, AF.Copy)
                cp_flip[0] += 1

            # ---- u-build: u[rc][:, i*P:(i+1)*P] = wdn-block.T @ xTn-block ----
            def u_build():
                for rc in range(RC):
                    for i in range(CI):
                        b = rc * 8 + i
                        rg = 32 * (b % 4)
                        lhsT = t_wdn[rg:rg + 16,
                                     128 * (b // 4):128 * (b // 4) + 128]
                        rhs = t_xtn[rg:rg + 16, P * (i // 4):P * (i // 4) + P]
                        ps = ppA.tile([128, P], FP32, tag="ps", name="ps")
                        nc.tensor.matmul(ps[:, 0:512], lhsT, rhs[:, 0:512],
                                         start=True, stop=True,
                                         tile_position=(rg, 0))
                        nc.tensor.matmul(ps[:, 512:P], lhsT, rhs[:, 512:P],
                                         start=True, stop=True,
                                         tile_position=(rg, 0))
                        psum_to_sbuf(t_u[rc][:, i * P:(i + 1) * P], ps[:])

            def s_accum(rc, d_grid_ap):
                """t_s[jm, q] += window of d (d viewed as [128, 21, 32])."""
                r, c = divmod(rc, KW)
                win = d_grid_ap[:, r:r + QR, c:c + QC]
                sview = t_s[:].rearrange("p (a b) -> p a b", b=QC)
                if rc == 0:
                    nc.vector.tensor_copy(sview, win)
                else:
                    with nc.allow_low_precision(reason="fp16 s accumulate"):
                        nc.vector.tensor_tensor(sview, sview, win, OP.add)

            def squeeze(it):
                """v = squeeze(s); writes vpads (it<2) or vout (it==2)."""
                # s*s overflows fp16 at later iterations: compute (s*sig)*s so
                # the fp16 products stay in range, and un-scale via 1/sig.
                sig = (1.0, 2.0 ** -8, 2.0 ** -12)[it]
                isig = 1.0 / sig
                s2 = wpool.tile([128, Q], FP16, tag="s2")
                nc.vector.scalar_tensor_tensor(s2[:], t_s[:], sig, t_s[:],
                                               OP.mult, OP.mult)
                sq_t = ppA.tile([128, P], FP32, tag="ps", name="sq_t")
                nc.tensor.matmul(sq_t[0:8, 0:512], t_o16[:], s2[:, 0:512],
                                 start=True, stop=True)
                nc.tensor.matmul(sq_t[0:8, 512:Q], t_o16[:], s2[:, 512:Q],
                                 start=True, stop=True)
                sq_ap = sq_t[0:8, 0:Q]        # = sig * true_sq
                t1 = tpool.tile([8, P], FP32, tag="tmp8")
                nc.vector.tensor_scalar(t1[:, 0:Q], sq_ap, isig, 1.0,
                                        OP.mult, OP.add)
                sqr = tpool.tile([8, P], FP32, tag="tmp8")
                nc.scalar.activation(sqr[:, 0:Q], sq_ap, AF.Sqrt, scale=isig,
                                     bias=t_eps[0:8, :])
                if it < R_NUM - 1:
                    # prefetch the Exp table for the upcoming softmax
                    nc.scalar.activation(t_dum[:], t_eps[0:8, :], AF.Exp)
                den = tpool.tile([8, P], FP32, tag="tmp8")
                nc.vector.tensor_tensor(den[:, 0:Q], t1[:, 0:Q], sqr[:, 0:Q],
                                        OP.mult)
                rec = tpool.tile([8, P], FP32, tag="tmp8")
                nc.vector.reciprocal_approx_fast(rec[:, 0:Q], den[:, 0:Q])
                f = tpool.tile([8, P], FP16, tag="tmp8f")
                nc.vector.scalar_tensor_tensor(f[:, 0:Q], sq_ap, isig,
                                               rec[:, 0:Q], OP.mult, OP.mult)
                fe_t = ppA.tile([128, P], FP32, tag="ps", name="fe_t")
                nc.tensor.matmul(fe_t[:, 0:512], t_dj[:], f[:, 0:512],
                                 start=True, stop=True)
                nc.tensor.matmul(fe_t[:, 512:Q], t_dj[:], f[:, 512:Q],
                                 start=True, stop=True)
                fe_ap = fe_t[:, 0:Q]
                if it == R_NUM - 1:
                    t_vo = wpool.tile([128, Q], FP32, tag="vo", name="vo2")
                    nc.vector.tensor_tensor(t_vo[:], t_s[:], fe_ap, OP.mult)
                    # split the store across 4 queues (128 one-row
                    # descriptors on one queue would cost ~9us)
                    for q, eng in enumerate((nc.sync, nc.scalar, nc.gpsimd,
                                             nc.sync)):
                        eng.dma_start(vout[32 * q:32 * (q + 1), :],
                                      t_vo[32 * q:32 * (q + 1), :])
                else:
                    for c in range(KW):
                        dst = t_vp[c][:, 2:2 + QR, c:c + QC]
                        nc.vector.tensor_tensor(
                            dst,
                            t_s[:].rearrange("p (a b) -> p a b", b=QC),
                            fe_ap.rearrange("p (a b) -> p a b", b=QC),
                            OP.mult)

            def agreement(nxt):
                """bT += sum_m u*vp per (rc, i); PE masked-ones matmuls.

                rc pairs (2t, 2t+1) land in rows 0:64 / 64:128 of one PSUM
                tile so each bT tile updates with a single 128-row add. The
                next softmax's exp (ACT) + row-sum (PE) for tile t are
                emitted as soon as bT[t] is final, overlapping phases."""
                shift = 0.0 if nxt < 2 else -8.0
                exs = [None] * NBT
                sume_t = ppB.tile([128, 1024], FP32, tag="sume")
                dps = None
                for rc in range(RC):
                    r, c = divmod(rc, KW)
                    t, row = rc // 2, (rc % 2) * 64
                    if row == 0:
                        dps = ppA.tile([128, P], FP32, tag="ps",
                                       name=f"dps{t}")
                    vslice = t_vp[c][:, 2 - r:2 - r + RROWS, 0:W]
                    vb = vslice.unsqueeze(1).broadcast_to([128, 4, RROWS, W])
                    for h in range(2):
                        prod = wpool.tile([128, 4 * P], FP16, tag="big",
                                          name=f"prod{rc}_{h}")
                        useg = t_u[rc][:, h * 4 * P:(h + 1) * 4 * P]
                        nc.vector.tensor_tensor(
                            prod[:].rearrange("p (i a b) -> p i a b", i=4, b=W),
                            useg.rearrange("p (i a b) -> p i a b", i=4, b=W),
                            vb, OP.mult)
                        for ih in range(4):
                            i = h * 4 + ih
                            lhsT = t_mo[:, i * 64:(i + 1) * 64]
                            seg = prod[:, ih * P:(ih + 1) * P]
                            nc.tensor.matmul(dps[row:row + 64, 0:512], lhsT,
                                             seg[:, 0:512],
                                             start=(i == 0), stop=(i == CI - 1))
                            nc.tensor.matmul(dps[row:row + 64, 512:P], lhsT,
                                             seg[:, 512:P],
                                             start=(i == 0), stop=(i == CI - 1))
                    if row == 64 or rc == RC - 1:
                        rows = 64 if rc == RC - 1 else 128
                        bslice = t_bt[t][0:rows, :]
                        nc.vector.tensor_tensor(bslice, bslice,
                                                dps[0:rows, :], OP.add)
                        ex = epool.tile([128, P], FP16, tag=f"ex{t}",
                                        name=f"ex{t}")
                        if shift == 0.0:
                            nc.scalar.activation(ex[:], t_bt[t][:], AF.Exp,
                                                 scale=8.0)
                        else:
                            nc.scalar.activation(ex[:], t_bt[t][:], AF.Exp,
                                                 scale=8.0, bias=t_shm8[:])
                        exs[t] = ex
                        lhsT = t_ms[:, t * 8:(t + 1) * 8]
                        nc.tensor.matmul(sume_t[0:8, 0:512], lhsT,
                                         ex[:, 0:512],
                                         start=(t == 0), stop=(t == NBT - 1))
                        nc.tensor.matmul(sume_t[0:8, 512:P], lhsT,
                                         ex[:, 512:P],
                                         start=(t == 0), stop=(t == NBT - 1))
                return exs, sume_t

            def softmax_rest_tile(it, exs, sume_t, t):
                """Per-tile tail of the softmax: c[t] = ex[t] * re[t].

                c = exp(8 bT) * 7200 / (sum + eps + 7128); exp + row-sums
                already ran inside agreement(). A constant logit shift keeps
                exp in fp16 range (it==2 logits reach ~17); it cancels
                exactly in c = 7200*ex/(sum + 7128*e^shift)."""
                if t == 0:
                    shift = 0.0 if it < 2 else -8.0
                    cadd = (J_ADD + EPS) * float(np.exp(shift))
                    # prefetch the Sqrt table for the upcoming squeeze
                    nc.scalar.activation(t_dum[:], t_eps[0:8, :], AF.Sqrt)
                    sumb = tpool.tile([8, P], FP32, tag="tmp8")
                    nc.vector.tensor_scalar_add(sumb[:], sume_t[0:8, 0:P],
                                                cadd)
                    rec = tpool.tile([8, P], FP32, tag="tmp8")
                    nc.vector.reciprocal_approx_fast(rec[:], sumb[:])
                    rec16 = tpool.tile([8, P], FP16, tag="tmp8f",
                                       name="rec16")
                    softmax_rest_tile.rec16 = rec16
                    nc.vector.tensor_copy(rec16[:], rec[:])
                rec16 = softmax_rest_tile.rec16
                re = ppA.tile([128, P], FP32, tag="ps", name="re")
                lhsT = t_dr[:, t * 128:(t + 1) * 128]
                nc.tensor.matmul(re[:, 0:512], lhsT, rec16[:, 0:512],
                                 start=True, stop=True)
                nc.tensor.matmul(re[:, 512:P], lhsT, rec16[:, 512:P],
                                 start=True, stop=True)
                # ACT drains re to fp16 so the c-mult runs at DVE 2x
                re16 = cppool.tile([128, P], FP16, tag="ce16", name="re16")
                nc.scalar.activation(re16[:], re[:], AF.Copy)
                nc.vector.tensor_tensor(t_c[t][:], exs[t][:], re16[:],
                                        OP.mult)

            def expand_unit(rc, i, ce_ps):
                """PE-expand c rows for slot i into a [128, P] PSUM tile."""
                t, row0 = rc // 2, (rc % 2) * 64
                r0 = row0 + i * 8
                r32 = (r0 // 32) * 32           # 32-aligned ldweights base
                g = (r0 - r32) // 8             # 8-row group within window
                lhsT = t_e128[r32:r32 + 32, g * 128:(g + 1) * 128]
                rhs = t_c[t][r32:r32 + 32, :]
                nc.tensor.matmul(ce_ps[:, 0:512], lhsT, rhs[:, 0:512],
                                 start=True, stop=True,
                                 tile_position=(r32, 0))
                nc.tensor.matmul(ce_ps[:, 512:P], lhsT, rhs[:, 512:P],
                                 start=True, stop=True,
                                 tile_position=(r32, 0))

            # per-i slot kinds in the d-pass:
            #  'a': ACT drains PSUM->fp16 SBUF, DVE fp16-2x product
            #  'd': DVE multiplies straight from PSUM (1x, but no drain op)
            #  'g': ACT drains, GpSimd does the product (PSUM is gp-illegal)
            CE_SLOT = ("a", "a", "d", "a", "g", "a", "a", "d")

            def d_pass_coeff(it, exs, sume_t):
                """s = sum_rc window(sum_i c*u): PE expands c rows (j -> jm)
                via a constant delta-matrix matmul into PSUM; ACT drains to
                fp16 SBUF; DVE fp16-2x products + in-place add tree. The
                per-tile softmax tail is interleaved so expansions for
                rc-pair t start right after c[t] is ready."""
                for rc in range(RC):
                    if rc % 2 == 0:
                        softmax_rest_tile(it, exs, sume_t, rc // 2)
                    cu = wpool.tile([128, 8 * P], FP16, tag="cu",
                                    name=f"cu{rc}")
                    for i in range(CI):
                        ce_ps = ppA.tile([128, P], FP32, tag="ps", name="ceps")
                        expand_unit(rc, i, ce_ps)
                        useg = t_u[rc][:, i * P:(i + 1) * P]
                        cuseg = cu[:, i * P:(i + 1) * P]
                        kind = CE_SLOT[i]
                        if kind == "d":
                            nc.vector.tensor_tensor(cuseg, useg, ce_ps[:],
                                                    OP.mult)
                        else:
                            ce16 = cppool.tile([128, P], FP16, tag="ce16")
                            nc.scalar.activation(ce16[:], ce_ps[:], AF.Copy)
                            eng = nc.gpsimd if kind == "g" else nc.vector
                            eng.tensor_tensor(cuseg, useg, ce16[:], OP.mult)
                    with nc.allow_low_precision(reason="fp16 routing tree"):
                        eng1 = nc.vector
                        eng1.tensor_tensor(cu[:, 0:4 * P], cu[:, 0:4 * P],
                                           cu[:, 4 * P:8 * P], OP.add)
                        nc.vector.tensor_tensor(cu[:, 0:2 * P], cu[:, 0:2 * P],
                                                cu[:, 2 * P:4 * P], OP.add)
                        nc.vector.tensor_tensor(cu[:, 0:P], cu[:, 0:P],
                                                cu[:, P:2 * P], OP.add)
                    s_accum(rc, cu[:, 0:P].rearrange("p (a b) -> p a b", b=W))

            if phases < 1:
                raise _PhaseStop
            # ================= iteration 0 =================
            # c == 1 exactly (b=0): s0 = sum_rc window(Wd[rc].T @ xT128)
            # fully on PE via strided window views of xT128, PSUM-accumulated
            # over all 9 taps (split into two banks: 10+9 output rows).
            # Emitted before the u-build so squeeze(0) overlaps it.
            xv = t_x128[:].rearrange("p (a b) -> p a b", b=W)
            s0 = ppB.tile([128, 1024], FP32, tag="sume")
            s0a = s0[:, 0:10 * QC]          # bank 0
            s0b = s0[:, 512:512 + 9 * QC]   # bank 1
            for rc in range(RC):
                r, c = divmod(rc, KW)
                lhsT = t_wd[:, rc * 128:(rc + 1) * 128]
                rhs_a = xv[:, r:r + 10, c:c + QC]
                rhs_b = xv[:, r + 10:r + QR, c:c + QC]
                nc.tensor.matmul(s0a, lhsT, rhs_a,
                                 start=(rc == 0), stop=(rc == RC - 1))
                nc.tensor.matmul(s0b, lhsT, rhs_b,
                                 start=(rc == 0), stop=(rc == RC - 1))
            nc.vector.tensor_copy(t_s[:, 0:10 * QC], s0a)
            nc.vector.tensor_copy(t_s[:, 10 * QC:Q], s0b)
            squeeze(0)
            if phases < 2:
                raise _PhaseStop
            u_build()
            exs, sume_t = agreement(1)
            if phases < 3:
                raise _PhaseStop

            # ================= iterations 1..2 =================
            for it in range(1, R_NUM):
                if phases < 3 + (it - 1) * 4 + 1:
                    raise _PhaseStop
                d_pass_coeff(it, exs, sume_t)
                squeeze(it)
                if phases < 3 + (it - 1) * 4 + 3:
                    raise _PhaseStop
                if it < R_NUM - 1:
                    exs, sume_t = agreement(2)

    nc.compile()
    return nc


_PROGRAM = None


def _get_program():
    global _PROGRAM
    if _PROGRAM is None:
        _PROGRAM = _build_program()
    return _PROGRAM


def _host_inputs(x, w):
    """Build the 8 per-core input maps."""
    wdn_p = np.zeros((128, 18 * 128), np.float16)
    wd_p = np.zeros((128, RC * 128), np.float16)
    wf = w.reshape(RC, CI, NI, CJ * NJ)              # [rc, i, n, jm]
    for rc in range(RC):
        for i in range(CI):
            b = rc * 8 + i
            wdn_p[32 * (b % 4):32 * (b % 4) + 16,
                  128 * (b // 4):128 * (b // 4) + 128] = wf[rc, i]
            wd_p[i * 16:(i + 1) * 16, rc * 128:(rc + 1) * 128] = wf[rc, i]
    mones = np.zeros((128, 8 * 64), np.float16)
    for i in range(CI):
        for j in range(CJ):
            mones[j * 16:(j + 1) * 16, i * 64 + i * 8 + j] = 1.0
    msum = np.zeros((128, NBT * 8), np.float16)
    dltr = np.zeros((8, NBT * 128), np.float16)
    for g in range(RC * CI * CJ):                    # g = rc*64 + i*8 + j
        i = (g % 64) // 8
        t, r = g // 128, g % 128
        msum[r, t * 8 + i] = 1.0
        dltr[i, t * 128 + r] = J_ALL
    e128p = np.zeros((128, 512), np.float16)
    for k in range(128):
        g = (k % 32) // 8
        j = k % 8
        e128p[k, g * 128 + j * 16:g * 128 + (j + 1) * 16] = 1.0
    ones16 = np.zeros((128, 8), np.float16)
    dltj = np.zeros((8, 128), np.float16)
    for j in range(CJ):
        ones16[j * 16:(j + 1) * 16, j] = 1.0
        dltj[j, j * 16:(j + 1) * 16] = 1.0

    shared = dict(wdn=wdn_p, wd=wd_p, mones=mones, msum=msum, dltr=dltr,
                  ones16=ones16, dltj=dltj, e128=e128p)

    in_maps = []
    for core in range(8):
        b, half = divmod(core, 2)
        r0 = 0 if half == 0 else H - RROWS
        xs = x[b, r0:r0 + RROWS].astype(np.float16)   # (21, 32, 8, 16)
        xt128 = np.ascontiguousarray(xs.reshape(P, CI * NI).T)
        xtn = np.zeros((128, 2 * P), np.float16)
        for i in range(CI):
            xtn[32 * (i % 4):32 * (i % 4) + 16,
                P * (i // 4):P * (i // 4) + P] = xs[:, :, i, :].reshape(P, NI).T
        m = dict(shared)
        m["xTn"] = xtn
        m["xT128"] = xt128
        in_maps.append(m)
    return in_maps


def _assemble(results):
    out = np.zeros((B, HJ, WJ, CJ, NJ), np.float32)
    for core, res in enumerate(results):
        b, half = divmod(core, 2)
        v = res["vout"].reshape(CJ, NJ, QR, QC).transpose(2, 3, 0, 1)
        if half == 0:
            out[b, 0:15] = v[0:15]
        else:
            out[b, 15:30] = v[4:19]
    return out


def run(x, w, trace=False):
    x = np.asarray(x, np.float32)
    w = np.asarray(w, np.float32)
    nc = _get_program()
    in_maps = _host_inputs(x, w)
    res = run_bass_kernel_spmd(nc, in_maps, core_ids=list(range(8)), trace=trace)
    return _assemble(res.results), res


def kernel(x, w):
    out, _ = run(x, w)
    return out

